# revision 5
# baseline (speedup 1.0000x reference)
"""MultiHeadDiffAttention kernel for 8 trn2 NeuronCores.

Sharding: tensor-parallel over heads (H=8, one head per core).
Per core (head h):
  qT/kT = Wq_h @ x.T   [128 feat, 2048 tok] per batch (bf16 matmuls, f32 accum)
  v     = x @ Wv_h.T   [2048 tok, 128 dh], augmented with a ones column
  scoresT[k, q] per diff-branch via row-packed PE matmuls (c=64, concurrent)
  exp on ScalarE (scores ~ N(0,1): no max subtraction needed)
  u = exp.T @ [v | 1]  -> attention numerator + softmax denominator in one matmul
  o = u1*r1 - dw*u2*r2 (per-partition scalars)
Then one AllToAll (2 MB) converts head-sharded o -> token-sharded o, so the
joint-head RMS reduction is local. norm_w and (1-dw) are folded into Wo on the
host. Each core emits output rows for its 512-token slice; host concatenates.
"""

import os
import sys

import numpy as np

if "/opt/trn_rl_repo" not in sys.path:
    sys.path.insert(0, "/opt/trn_rl_repo")

B, S, E, H = 2, 2048, 1024, 8
DH = E // H          # 128
F = DH // 2          # 64
P = 128              # partitions
NCORES = 8
TOK = B * S          # 4096
TPC = TOK // NCORES  # 512 tokens per core (phase-3 slice)
EC = E // P          # 8 e-chunks
KC = S // P          # 16 k-chunks per batch
QBS = 512            # q-block size
QB = S // QBS        # 4 q-blocks per batch
QT = QBS // P        # 4 q-tiles per q-block
TT = TPC // P        # 4 token tiles in phase 3
EPS = float(np.finfo(np.float32).eps)

LAST_RESULTS = None  # BassKernelResults of the most recent run (test.py reads this)

_NC_CACHE: dict = {}


def _build(dw: float):
    import concourse.bass as bass
    import concourse.mybir as mybir
    import concourse.tile as tile
    from concourse import bacc, masks

    dt = mybir.dt
    AF = mybir.ActivationFunctionType

    nc = bacc.Bacc("TRN2", target_bir_lowering=False, debug=False, num_devices=NCORES)

    xT_d = nc.dram_tensor("xT", [B, E, S], dt.bfloat16, kind="ExternalInput")
    wqT_d = nc.dram_tensor("wqT", [E, DH], dt.bfloat16, kind="ExternalInput")
    wkT_d = nc.dram_tensor("wkT", [E, DH], dt.bfloat16, kind="ExternalInput")
    wvT_d = nc.dram_tensor("wvT", [E, DH], dt.bfloat16, kind="ExternalInput")
    woT_d = nc.dram_tensor("woT", [E, E], dt.bfloat16, kind="ExternalInput")
    out_d = nc.dram_tensor("out", [TPC, E], dt.float32, kind="ExternalOutput")

    with tile.TileContext(nc) as tc:
        with (
            tc.tile_pool(name="consts", bufs=1) as consts,
            tc.tile_pool(name="xt", bufs=1) as xtp,
            tc.tile_pool(name="qk", bufs=2) as qkp,
            tc.tile_pool(name="vp", bufs=2) as vp,
            tc.tile_pool(name="expp", bufs=2) as expp,
            tc.tile_pool(name="osb", bufs=2) as osb,
            tc.tile_pool(name="small", bufs=8) as small,
            tc.tile_pool(name="mid", bufs=4) as mid,
            tc.tile_pool(name="p3", bufs=2) as p3,
            tc.tile_pool(name="dram", bufs=1, space="DRAM") as dram,
            tc.tile_pool(name="psA", bufs=3, space="PSUM") as psA,
            tc.tile_pool(name="psB", bufs=4, space="PSUM") as psB,
            tc.tile_pool(name="psC", bufs=1, space="PSUM") as psC,
        ):
            ident = consts.tile([P, P], dt.bfloat16, tag="ident")
            masks.make_identity(nc, ident)
            eps_t = consts.tile([P, 1], dt.float32, tag="eps")
            nc.vector.memset(eps_t, EPS)

            wq_sb = consts.tile([P, EC, DH], dt.bfloat16, tag="wq")
            wk_sb = consts.tile([P, EC, DH], dt.bfloat16, tag="wk")
            wv_sb = consts.tile([P, EC, DH], dt.bfloat16, tag="wv")
            for w_sb, w_d in ((wq_sb, wqT_d), (wk_sb, wkT_d), (wv_sb, wvT_d)):
                nc.sync.dma_start(
                    out=w_sb, in_=w_d.rearrange("(c p) d -> p c d", p=P)
                )
            wo_sb = consts.tile([P, EC, E], dt.bfloat16, tag="wo")
            nc.sync.dma_start(out=wo_sb, in_=woT_d.rearrange("(c p) e -> p c e", p=P))

            a2a_in = dram.tile([TOK, DH], dt.bfloat16, tag="a2a_in")
            a2a_out = dram.tile([TOK, DH], dt.bfloat16, tag="a2a_out")
            a2a_in_v = a2a_in.rearrange(
                "(b qb q p) d -> b qb p q d", b=B, qb=QB, p=P
            )  # [B, QB, P, QT, DH]; token = b*S + qb*QBS + qt*P + p

            for b in range(B):
                xt = xtp.tile([P, EC, S], dt.bfloat16, tag="xt")
                nc.sync.dma_start(
                    out=xt, in_=xT_d[b].rearrange("(c p) t -> p c t", p=P)
                )

                # --- projections ---
                qT = qkp.tile([P, S], dt.bfloat16, tag="qT")
                kT = qkp.tile([P, S], dt.bfloat16, tag="kT")
                for w_sb, dst in ((wq_sb, qT), (wk_sb, kT)):
                    for tb in range(S // 512):
                        ps = psA.tile([P, 512], dt.float32, tag="sc")
                        for ec in range(EC):
                            nc.tensor.matmul(
                                ps,
                                lhsT=w_sb[:, ec, :],
                                rhs=xt[:, ec, tb * 512 : (tb + 1) * 512],
                                start=(ec == 0),
                                stop=(ec == EC - 1),
                            )
                        nc.vector.tensor_copy(dst[:, tb * 512 : (tb + 1) * 512], ps)

                v = vp.tile([P, KC, DH + 1], dt.bfloat16, tag="v")
                nc.vector.memset(v[:, :, DH : DH + 1], 1.0)
                for kt in range(KC):
                    ps = psA.tile([P, 512], dt.float32, tag="sc")
                    for ec in range(EC):
                        nc.tensor.matmul(
                            ps[:, :DH],
                            lhsT=xt[:, ec, kt * P : (kt + 1) * P],
                            rhs=wv_sb[:, ec, :],
                            start=(ec == 0),
                            stop=(ec == EC - 1),
                        )
                    nc.vector.tensor_copy(v[:, kt, :DH], ps[:, :DH])

                # --- attention ---
                for qb in range(QB):
                    qs = slice(qb * QBS, (qb + 1) * QBS)
                    e1 = expp.tile([P, KC, QBS], dt.bfloat16, tag="e1")
                    e2 = expp.tile([P, KC, QBS], dt.bfloat16, tag="e2")
                    for kt in range(KC):
                        ks = slice(kt * P, (kt + 1) * P)
                        s1 = psA.tile([P, QBS], dt.float32, tag="sc")
                        s2 = psA.tile([P, QBS], dt.float32, tag="sc")
                        nc.tensor.matmul(s1, lhsT=kT[0:F, ks], rhs=qT[0:F, qs])
                        nc.tensor.matmul(s2, lhsT=kT[F:P, ks], rhs=qT[F:P, qs])
                        nc.scalar.activation(e1[:, kt, :], s1, AF.Exp, scale=F**-0.5)
                        nc.scalar.activation(e2[:, kt, :], s2, AF.Exp, scale=F**-0.5)

                    o_t = osb.tile([P, QT, DH], dt.bfloat16, tag="o_t")
                    for qt in range(QT):
                        qts = slice(qt * P, (qt + 1) * P)
                        u1 = psB.tile([P, DH + 1], dt.float32, tag="u")
                        u2 = psB.tile([P, DH + 1], dt.float32, tag="u")
                        for kt in range(KC):
                            nc.tensor.matmul(
                                u1,
                                lhsT=e1[:, kt, qts],
                                rhs=v[:, kt, :],
                                start=(kt == 0),
                                stop=(kt == KC - 1),
                            )
                        for kt in range(KC):
                            nc.tensor.matmul(
                                u2,
                                lhsT=e2[:, kt, qts],
                                rhs=v[:, kt, :],
                                start=(kt == 0),
                                stop=(kt == KC - 1),
                            )
                        r1 = small.tile([P, 1], dt.float32, tag="r1")
                        r2 = small.tile([P, 1], dt.float32, tag="r2")
                        nc.vector.reciprocal(r1, u1[:, DH : DH + 1])
                        nc.vector.reciprocal(r2, u2[:, DH : DH + 1])
                        r2n = small.tile([P, 1], dt.float32, tag="r2n")
                        nc.vector.tensor_scalar_mul(r2n, r2, -dw)
                        t2 = mid.tile([P, DH], dt.float32, tag="t2")
                        nc.vector.tensor_scalar_mul(t2, u2[:, :DH], r2n)
                        o1 = mid.tile([P, DH], dt.float32, tag="o1")
                        nc.scalar.mul(o1, u1[:, :DH], mul=r1)
                        nc.vector.tensor_add(o_t[:, qt, :], o1, t2)
                    nc.sync.dma_start(out=a2a_in_v[b, qb], in_=o_t)

            nc.gpsimd.collective_compute(
                "AllToAll",
                mybir.AluOpType.bypass,
                replica_groups=[list(range(NCORES))],
                ins=[a2a_in.opt()],
                outs=[a2a_out.opt()],
            )

            # --- phase 3: RMS norm + output projection on my 512-token slice ---
            a2a_out_v = a2a_out.rearrange(
                "(h q p) d -> q p h d", h=H, p=P
            )  # [TT, P, H, DH]
            for tt in range(TT):
                o2 = p3.tile([P, H, DH], dt.bfloat16, tag="o2")
                nc.sync.dma_start(out=o2, in_=a2a_out_v[tt])
                sq = p3.tile([P, H, DH], dt.bfloat16, tag="sq")
                ssq = small.tile([P, 1], dt.float32, tag="ssq")
                nc.scalar.activation(sq, o2, AF.Square, accum_out=ssq)
                sroot = small.tile([P, 1], dt.float32, tag="sroot")
                nc.scalar.activation(sroot, ssq, AF.Sqrt, scale=1.0 / E, bias=eps_t)
                rms = small.tile([P, 1], dt.float32, tag="rms")
                nc.vector.reciprocal(rms, sroot)
                nrm = p3.tile([P, H, DH], dt.bfloat16, tag="nrm")
                nc.vector.tensor_scalar_mul(nrm, o2, rms)
                nT = p3.tile([P, EC, P], dt.bfloat16, tag="nT")
                for fc in range(EC):
                    tp = psC.tile([P, P], dt.bfloat16, tag="tp")
                    nc.tensor.transpose(tp, nrm[:, fc, :], ident)
                    nc.vector.tensor_copy(nT[:, fc, :], tp)
                out_sb = p3.tile([P, E], dt.float32, tag="out_sb")
                for nb in range(E // 512):
                    ps = psA.tile([P, 512], dt.float32, tag="sc")
                    for fc in range(EC):
                        nc.tensor.matmul(
                            ps,
                            lhsT=nT[:, fc, :],
                            rhs=wo_sb[:, fc, nb * 512 : (nb + 1) * 512],
                            start=(fc == 0),
                            stop=(fc == EC - 1),
                        )
                    nc.vector.tensor_copy(out_sb[:, nb * 512 : (nb + 1) * 512], ps)
                nc.sync.dma_start(out=out_d[tt * P : (tt + 1) * P, :], in_=out_sb)

    nc.compile()
    return nc


def _get_nc(dw: float):
    key = round(float(dw), 9)
    if key not in _NC_CACHE:
        _NC_CACHE[key] = _build(float(dw))
    return _NC_CACHE[key]


def kernel(x, Wq, Wk, Wv, norm_w, Wo, bo, diff_weight):
    import ml_dtypes

    from concourse.bass_utils import run_bass_kernel_spmd

    global LAST_RESULTS

    bf16 = ml_dtypes.bfloat16
    x = np.asarray(x, dtype=np.float32)
    Wq = np.asarray(Wq, dtype=np.float32)
    Wk = np.asarray(Wk, dtype=np.float32)
    Wv = np.asarray(Wv, dtype=np.float32)
    Wo = np.asarray(Wo, dtype=np.float32)
    norm_w = np.asarray(norm_w, dtype=np.float32)
    bo = np.asarray(bo, dtype=np.float32)
    dw = float(np.asarray(diff_weight))

    nc = _get_nc(dw)

    xT = np.ascontiguousarray(x.transpose(0, 2, 1)).astype(bf16)  # [B, E, S]
    woT = np.ascontiguousarray(
        (Wo * norm_w.reshape(-1)[None, :] * (1.0 - dw)).T
    ).astype(bf16)  # [E(feat), E(out)]

    in_maps = []
    for h in range(NCORES):
        rows = slice(h * DH, (h + 1) * DH)
        in_maps.append(
            {
                "xT": xT,
                "wqT": np.ascontiguousarray(Wq[rows, :].T).astype(bf16),
                "wkT": np.ascontiguousarray(Wk[rows, :].T).astype(bf16),
                "wvT": np.ascontiguousarray(Wv[rows, :].T).astype(bf16),
                "woT": woT,
            }
        )

    res = run_bass_kernel_spmd(
        nc,
        in_maps,
        core_ids=list(range(NCORES)),
        trace=bool(os.environ.get("KERNEL_TRACE")),
    )
    LAST_RESULTS = res

    full = np.concatenate([res.results[c]["out"] for c in range(NCORES)], axis=0)
    full = full + (1.0 - dw) * bo[None, :]
    return full.reshape(B, S, E).astype(np.float32)


if __name__ == "__main__":
    rng = np.random.default_rng(0)
    sc = E**-0.5
    ins = {
        "x": rng.standard_normal((B, S, E), dtype=np.float32),
        "Wq": rng.standard_normal((E, E), dtype=np.float32) * sc,
        "Wk": rng.standard_normal((E, E), dtype=np.float32) * sc,
        "Wv": rng.standard_normal((E, E), dtype=np.float32) * sc,
        "norm_w": np.ones((H, DH), dtype=np.float32),
        "Wo": rng.standard_normal((E, E), dtype=np.float32) * sc,
        "bo": np.zeros((E,), dtype=np.float32),
        "diff_weight": np.float32(0.2),
    }
    out = kernel(**ins)
    print("out", out.shape, out.dtype, float(np.abs(out).max()))
